# revision 1
# baseline (speedup 1.0000x reference)
"""Trainium2 Bass kernel: embedding lookup -> 2-layer MLP -> softmax(32000).

Computation (reference):
    h  = relu(W1[:, x].T + b1)          # [N, 256] embedding gather
    h2 = relu(h @ W2.T + b2)            # [N, 512]
    p  = softmax(h2 @ W3.T + b3)        # [N, 32000]

Sharding: 8-way tensor parallel over the vocab dim of W3/b3 (4000 cols per
core). Every core computes h2 for all 8192 tokens (cheap, replicated), its
4000-wide logit slice, exp() kept resident in SBUF, partial row-sums
all-reduced across the 8 cores, then scales in place and writes its
[8192, 4000] output slice once.

Pipeline per 512-token group g: B(g) logits+exp, C(g) collective kick,
A(g+1) gather/transpose/h2 prefetch, D(g) scale+store. The embedding path
runs in fp16 (gather + DMA-xbar transpose, no PE transposes); logit matmuls
are fp16 with fp32 PSUM accumulation; b3 enters via a K=1 ones-row matmul;
exp/softmax stay fp32.
"""

import numpy as np

N_CORES = 8
N_TOK = 8192
VOCAB = 32000
H1 = 256
H2 = 512
VS = VOCAB // N_CORES          # 4000 vocab cols per core
BLK = 128                      # tokens per block (partition dim)
GROUP = 512                    # tokens per group (one collective per group)
BPG = GROUP // BLK             # 4 blocks per group
NG = N_TOK // GROUP            # 16 groups
VT = 500                       # vocab tile (one psum bank)
NVT = VS // VT                 # 8 vocab tiles per core

_compiled = None


def _build():
    import concourse.bass as bass
    import concourse.bacc as bacc
    import concourse.tile as tile
    from concourse import mybir

    f32 = mybir.dt.float32
    f16 = mybir.dt.float16
    i32 = mybir.dt.int32

    nc = bacc.Bacc("TRN2", target_bir_lowering=False, debug=False,
                   enable_asserts=True, num_devices=N_CORES)

    E_d = nc.dram_tensor("E", [VOCAB, H1], f16, kind="ExternalInput").ap()
    XT_d = nc.dram_tensor("XT", [BLK, N_TOK // BLK], i32, kind="ExternalInput").ap()
    W2_d = nc.dram_tensor("W2TP", [128, 1024], f16, kind="ExternalInput").ap()
    B2_d = nc.dram_tensor("B2T", [128, 4], f32, kind="ExternalInput").ap()
    W3_d = nc.dram_tensor("W3TP", [128, 4 * VS], f16, kind="ExternalInput").ap()
    EB3_d = nc.dram_tensor("EB3", [128, VS], f16, kind="ExternalInput").ap()
    OUT_d = nc.dram_tensor("OUT", [N_TOK, VS], f16, kind="ExternalOutput").ap()

    with tile.TileContext(nc) as tc:
        with (
            tc.tile_pool(name="const", bufs=1) as cp,
            tc.tile_pool(name="h1p", bufs=4) as h1p,
            tc.tile_pool(name="h1Tp", bufs=2) as h1Tp,
            tc.tile_pool(name="h2Tp", bufs=2) as h2Tp,
            tc.tile_pool(name="Up", bufs=4) as Up,
            tc.tile_pool(name="sgp", bufs=4) as sgp,
            tc.tile_pool(name="Sgp", bufs=4) as Sgp,
            tc.tile_pool(name="recp", bufs=4) as recp,
            tc.tile_pool(name="php", bufs=2, space="PSUM") as php,
            tc.tile_pool(name="plp", bufs=3, space="PSUM") as plp,
            tc.tile_pool(name="dramp", bufs=4, space="DRAM") as dramp,
        ):
            xt = cp.tile([BLK, N_TOK // BLK], i32)
            nc.sync.dma_start(xt[:], XT_d[:])
            w2t = cp.tile([128, 1024], f16)
            nc.sync.dma_start(w2t[:], W2_d[:])
            b2t = cp.tile([128, 4], f32)
            nc.sync.dma_start(b2t[:], B2_d[:])
            w3t = cp.tile([128, 4 * VS], f16)
            nc.sync.dma_start(w3t[:], W3_d[:])
            eb3 = cp.tile([128, VS], f16)
            nc.sync.dma_start(eb3[:], EB3_d[:])

            def phaseA(g):
                """gather + transpose + h2 for group g; returns h2T tile."""
                h1T = h1Tp.tile([128, 2 * GROUP], f16, tag="h1T",
                                name=f"h1T_{g}")
                h1T3 = h1T[:].rearrange("p (c t) -> p c t", c=2)
                for b in range(BPG):
                    h1 = h1p.tile([128, H1], f16, tag="h1", name=f"h1_{g}_{b}")
                    col = g * BPG + b
                    nc.gpsimd.indirect_dma_start(
                        out=h1[:], out_offset=None, in_=E_d[:],
                        in_offset=bass.IndirectOffsetOnAxis(
                            ap=xt[:, col:col + 1], axis=0),
                    )
                    nc.sync.dma_start_transpose(
                        h1T3[:, :, b * BLK:(b + 1) * BLK], h1[:])
                h2T = h2Tp.tile([128, 4 * GROUP], f16, tag="h2T",
                                name=f"h2T_{g}")
                for fc in range(4):
                    ph_ = php.tile([128, GROUP], f32, tag="ph",
                                   name=f"ph_{g}_{fc}")
                    for kc in range(2):
                        nc.tensor.matmul(
                            ph_[:],
                            lhsT=w2t[:, (fc * 2 + kc) * 128:(fc * 2 + kc + 1) * 128],
                            rhs=h1T[:, kc * GROUP:(kc + 1) * GROUP],
                            start=(kc == 0), stop=(kc == 1))
                    # h2T = relu(psum + b2) on DVE (keeps ACT exp-only)
                    nc.vector.tensor_scalar(
                        out=h2T[:, fc * GROUP:(fc + 1) * GROUP],
                        in0=ph_[:], scalar1=b2t[:, fc:fc + 1], scalar2=0.0,
                        op0=mybir.AluOpType.add, op1=mybir.AluOpType.max)
                return h2T

            def phaseB(g, h2T):
                """logits + exp for group g; returns (U, sg)."""
                U = Up.tile([128, BPG * VS], f16, tag="U", name=f"U_{g}")
                sg = sgp.tile([128, BPG], f32, tag="sg", name=f"sg_{g}")
                for b in range(BPG):
                    # 4 psum tiles of [128, 1024] (2 banks each); vocab tiles
                    # are 512-wide (bank-aligned), last one 416
                    for pr in range(4):
                        c0 = pr * 1024
                        w1 = 512 if pr < 3 else VS - c0 - 512
                        t_ = plp.tile([128, 1024], f32, tag="pl",
                                      name=f"pl_{g}_{b}_{pr}")
                        for fc in range(4):
                            lhs = h2T[:, fc * GROUP + b * BLK:
                                      fc * GROUP + (b + 1) * BLK]
                            for off, w in ((0, 512), (512, w1)):
                                nc.tensor.matmul(
                                    t_[:, off:off + w], lhsT=lhs,
                                    rhs=w3t[:, fc * VS + c0 + off:
                                            fc * VS + c0 + off + w],
                                    start=(fc == 0), stop=(fc == 3))
                        nc.scalar.activation(
                            U[:, b * VS + c0:b * VS + c0 + 512 + w1],
                            t_[:, :512 + w1],
                            mybir.ActivationFunctionType.Exp)
                    # U *= exp(b3), partial sums as a side effect
                    nc.vector.scalar_tensor_tensor(
                        out=U[:, b * VS:(b + 1) * VS],
                        in0=U[:, b * VS:(b + 1) * VS],
                        scalar=1.0, in1=eb3[:],
                        op0=mybir.AluOpType.mult_bypass
                        if hasattr(mybir.AluOpType, "mult_bypass")
                        else mybir.AluOpType.bypass,
                        op1=mybir.AluOpType.mult,
                        accum_out=sg[:, b:b + 1])
                return U, sg

            def phaseC(g, sg):
                """all-reduce kick; returns closure to fetch recip."""
                cin = dramp.tile([128, BPG], f32, tag="cin", name=f"cin_{g}")
                cout = dramp.tile([128, BPG], f32, tag="cout", name=f"cout_{g}")
                nc.gpsimd.dma_start(cin[:], sg[:])
                nc.gpsimd.collective_compute(
                    "AllReduce", mybir.AluOpType.add,
                    replica_groups=[list(range(N_CORES))],
                    ins=[cin.opt()], outs=[cout.opt()])
                return cout

            def phaseC2(g, cout):
                Sg = Sgp.tile([128, BPG], f32, tag="Sg", name=f"Sg_{g}")
                nc.gpsimd.dma_start(Sg[:], cout[:])
                rec = recp.tile([128, BPG], f32, tag="rec", name=f"rec_{g}")
                nc.vector.reciprocal(rec[:], Sg[:])
                nc.vector.tensor_scalar_mul(rec[:], rec[:], 1024.0)
                return rec

            def phaseD(g, U, rec):
                tok0 = g * GROUP
                for b in range(BPG):
                    nc.vector.tensor_scalar_mul(
                        U[:, b * VS:(b + 1) * VS],
                        U[:, b * VS:(b + 1) * VS], rec[:, b:b + 1])
                    nc.sync.dma_start(
                        OUT_d[tok0 + b * BLK: tok0 + (b + 1) * BLK, :],
                        U[:, b * VS:(b + 1) * VS])

            h2T = phaseA(0)
            for g in range(NG):
                U, sg = phaseB(g, h2T)
                cout = phaseC(g, sg)
                if g + 1 < NG:
                    h2T = phaseA(g + 1)
                rec = phaseC2(g, cout)
                phaseD(g, U, rec)

    nc.compile()
    return nc


def kernel(**inputs) -> np.ndarray:
    out, _ = _run(inputs)
    return out


def _run(inputs, trace: bool = False, **run_kwargs):
    global _compiled
    from concourse import bass_utils

    x = np.asarray(inputs["x"]).astype(np.int32)
    W1 = np.asarray(inputs["W1"], dtype=np.float32)
    b1 = np.asarray(inputs["b1"], dtype=np.float32)
    W2 = np.asarray(inputs["W2"], dtype=np.float32)
    b2 = np.asarray(inputs["b2"], dtype=np.float32)
    W3 = np.asarray(inputs["W3"], dtype=np.float32)
    b3 = np.asarray(inputs["b3"], dtype=np.float32)

    # host-side packing
    E = np.maximum(W1.T + b1[None, :], 0.0).astype(np.float16)  # [32000, 256]
    XT = np.ascontiguousarray(x.reshape(N_TOK // BLK, BLK).T)   # [128, 64]
    W2T = np.ascontiguousarray(W2.T)                            # [256, 512]
    w2chunks = [W2T[kc * 128:(kc + 1) * 128, fc * 128:(fc + 1) * 128]
                for fc in range(4) for kc in range(2)]
    W2TP = np.ascontiguousarray(
        np.concatenate(w2chunks, axis=1)).astype(np.float16)    # [128, 1024]
    B2T = np.ascontiguousarray(b2.reshape(4, 128).T)            # [128, 4]
    W3T = np.ascontiguousarray(W3.T)                            # [512, 32000]

    if _compiled is None:
        _compiled = _build()
    nc = _compiled

    in_maps = []
    for c in range(N_CORES):
        sl = slice(c * VS, (c + 1) * VS)
        w3c = W3T[:, sl].astype(np.float16)                     # [512, 4000]
        W3TP = np.ascontiguousarray(
            np.concatenate([w3c[k * 128:(k + 1) * 128] for k in range(4)],
                           axis=1))                             # [128, 16000]
        EB3 = np.ascontiguousarray(
            np.tile(np.exp(b3[sl]).astype(np.float16)[None, :], (128, 1)))
        in_maps.append({
            "E": E, "XT": XT, "W2TP": W2TP, "B2T": B2T,
            "W3TP": W3TP, "EB3": EB3,
        })

    res = bass_utils.run_bass_kernel_spmd(
        nc, in_maps, core_ids=list(range(N_CORES)), trace=trace, **run_kwargs)
    out = np.concatenate([res.results[c]["OUT"] for c in range(N_CORES)],
                         axis=1)
    return out.astype(np.float32) * np.float32(1.0 / 1024.0), res


if __name__ == "__main__":
    d = np.load("/root/problem/inputs_cache.npz")
    out = kernel(**{k: d[k] for k in d.files})
    ref = np.load("/root/problem/ref_cache.npy")
    diff = out - ref
    print("relL2:", np.linalg.norm(diff) / np.linalg.norm(ref))
    print("relmax:", np.abs(diff).max() / ref.max())

